# revision 14
# baseline (speedup 1.0000x reference)
"""Trainium2 Bass kernel for nn_CausalSelfAttention_67894843015857.

Full inputs -> full output. Sharding: 8 cores = 2 batches x 4 head-groups
(4 heads each). bf16 data paths everywhere (matmuls 1 cy/row at any free
size in the cost model, DVE 2x modes, half DMA bytes); f32 PSUM accumulate.

Per core:
  - qk projections (W stationary, x^T moving) -> qkT[m] in [dim, t] bf16,
    bias applied by Pool(tensor_scalar_add) during PSUM->SBUF drain
  - RoPE in-place: partition-rotated copy qsh built by 6 wide DMAs with the
    fork-channel overwrites folded in (cos rows 63/127 = 0, nsin rows = 1,
    qsh rows 63/127 <- ones / cumulative_scores), then 3 DVE ops
  - v projection -> V'' [t, 4*65] bf16 scaled by exp(cs)*padmask with a
    ones-column (softmax denominator) written once up front
  - attention chunk-outer (512-wide q chunks), TRANSPOSED scores S^T in
    [128,1024] PSUM pair tiles; diagonal quad packed contiguously
    ([512|384|256|128] cols) so exp covers exactly the causal region with
    2 instructions; 0/1 packed masks on DVE; PV accumulates [65, 512]
  - softmax denominators DMA'd from PSUM row 64 -> partition_broadcast
    (Pool) -> reciprocal (DVE) -> in-place scale of yT (deferred norm)
  - output projection per 4-t-tile block, pipelined one chunk behind
    attention; PSUM drained to bf16 SBUF by Pool/DVE, DMA'd out
Host: reduces the 4 per-batch bf16 partials in f32 and adds b_proj.
"""
import numpy as np
import ml_dtypes

import concourse.bacc as bacc
import concourse.mybir as mybir
import concourse.tile as tile
from concourse.bass_utils import run_bass_kernel_spmd

F32 = mybir.dt.float32
BF16 = mybir.dt.bfloat16
AF = mybir.ActivationFunctionType

P = 128
T = 2048
C = 1024
NKT = C // P          # 8 contraction tiles over the embedding dim
NT = T // P           # 16 t-tiles
SCALE = 0.125         # 1/sqrt(64)
CH = 512              # q-chunk width
BF = ml_dtypes.bfloat16

_NC_CACHE = {}


def build_nc():
    if "nc" in _NC_CACHE:
        return _NC_CACHE["nc"]
    nc = bacc.Bacc("TRN2", target_bir_lowering=False, debug=False)

    def din(name, shape, dt=BF16):
        return nc.dram_tensor(name, shape, dt, kind="ExternalInput").ap()

    xt_d = din("xt", [C, T])            # x[b].T
    wqk_d = din("wqk", [C, 512])        # col tiles: q01 | q23 | k01 | k23
    wv_d = din("wv", [C, 256])
    wp_d = din("wp", [256, C])
    bqk_d = din("bqk", [P, 4], F32)     # col m: bias for qkT[m] partitions
    bv_d = din("bv", [1, 256])
    cos_d = din("cos2", [P, T])         # rows 63,127 zeroed
    nsin_d = din("nsin", [P, T])        # rows 63,127 = 1
    ones_d = din("ones", [1, T])
    cs_d = din("csrow", [1, T])         # cumulative_scores[b]
    vs_d = din("vscale", [P, NT], F32)  # exp(cs)*pmbin, t-tiled columns
    oc_d = din("onec", [P, NT])         # pmbin, t-tiled columns
    dmA_d = din("dmaskA", [P, 896])     # packed diag mask, tiles (d0,d1)
    dmB_d = din("dmaskB", [P, 384])     # packed diag mask, tiles (d2,d3)
    out_d = nc.dram_tensor("outp", [T, C], BF16, kind="ExternalOutput").ap()

    with tile.TileContext(nc) as tc:
        with tc.tile_pool(name="const", bufs=1) as pc, \
             tc.tile_pool(name="persist", bufs=1) as pp:
            bqk_sb = pc.tile([P, 4], F32, name="bqk_sb")
            nc.sync.dma_start(bqk_sb[:], bqk_d[:])
            bv_sb = pc.tile([1, 256], BF16, name="bv_sb")
            nc.sync.dma_start(bv_sb[:], bv_d[:])
            ones_sb = pc.tile([1, T], BF16, name="ones_sb")
            nc.sync.dma_start(ones_sb[:], ones_d[:])
            cs_sb = pc.tile([1, T], BF16, name="cs_sb")
            nc.sync.dma_start(cs_sb[:], cs_d[:])
            vs_sb = pc.tile([P, NT], F32, name="vs_sb")
            nc.sync.dma_start(vs_sb[:], vs_d[:])
            oc_sb = pc.tile([P, NT], BF16, name="oc_sb")
            nc.sync.dma_start(oc_sb[:], oc_d[:])
            dmA_sb = pc.tile([P, 896], BF16, name="dmA_sb")
            nc.sync.dma_start(dmA_sb[:], dmA_d[:])
            dmB_sb = pc.tile([P, 384], BF16, name="dmB_sb")
            nc.sync.dma_start(dmB_sb[:], dmB_d[:])

            qk_t = [pp.tile([P, T], BF16, name=f"qkt{m}") for m in range(4)]
            vv = pp.tile([P, NT * 260], BF16, name="vv")
            yt = [pp.tile([P, T], BF16, name=f"yt{i}") for i in range(2)]
            wp_sb = pp.tile([P, 2 * C], BF16, name="wp_sb")

            # ones-columns of V'' (softmax denominator weights) once for all
            vv4 = vv.rearrange("p (t h x) -> p t h x", h=4, x=65)
            nc.vector.tensor_copy(
                vv4[:, :, :, 64:65],
                oc_sb[:, :, None, None].to_broadcast((P, NT, 4, 1)))

            pqsh = tc.alloc_tile_pool(name="qshp", bufs=2)
            with tc.tile_pool(name="load", bufs=1) as pl, \
                 tc.tile_pool(name="qkps", bufs=1, space="PSUM") as qkps, \
                 tc.tile_pool(name="vps", bufs=2, space="PSUM") as vpsp:
                xt = pl.tile([P, NKT * T], BF16, name="xt_sb")
                wqk = pl.tile([P, NKT * 512], BF16, name="wqk_sb")
                wv = pl.tile([P, NKT * 256], BF16, name="wv_sb")
                for k in range(NKT):
                    nc.sync.dma_start(wqk[:, k * 512:(k + 1) * 512],
                                      wqk_d[k * P:(k + 1) * P, :])
                    nc.sync.dma_start(xt[:, k * T:(k + 1) * T],
                                      xt_d[k * P:(k + 1) * P, :])
                nc.sync.dma_start(
                    wv.rearrange("p (k c) -> p k c", c=256),
                    wv_d.rearrange("(k p) c -> p k c", p=P))
                cos_sb = pl.tile([P, T], BF16, name="cos_sb")
                nc.sync.dma_start(cos_sb[:], cos_d[:])
                nsin_sb = pl.tile([P, T], BF16, name="nsin_sb")
                nc.sync.dma_start(nsin_sb[:], nsin_d[:])
                nc.sync.dma_start(
                    wp_sb.rearrange("p (k c) -> p k c", c=C),
                    wp_d.rearrange("(k p) c -> p k c", p=P))

                def qk_group(m):
                    """q/k projection + Pool bias-drain + RoPE."""
                    pss = [qkps.tile([P, CH], F32, name=f"qkps{m}_{n}",
                                     tag=f"qk{n}") for n in range(4)]
                    for k in range(NKT):
                        for n in range(4):
                            nc.tensor.matmul(
                                pss[n][:],
                                lhsT=wqk[:, k * 512 + m * P: k * 512 + (m + 1) * P],
                                rhs=xt[:, k * T + n * CH: k * T + (n + 1) * CH],
                                start=(k == 0), stop=(k == NKT - 1))
                    for n in range(4):
                        nc.scalar.activation(
                            qk_t[m][:, n * CH:(n + 1) * CH], pss[n][:],
                            AF.Identity, bias=bqk_sb[:, m:m + 1])
                    # RoPE: partition-rotated copy with fork rows folded in
                    qsh = pqsh.tile([P, T], BF16, name=f"qsh{m}", tag="qsh")
                    fork = ones_sb if m < 2 else cs_sb
                    nc.sync.dma_start(qsh[0:32, :], qk_t[m][32:64, :])
                    nc.sync.dma_start(qsh[32:63, :], qk_t[m][0:31, :])
                    nc.sync.dma_start(qsh[63:64, :], fork[0:1, :])
                    nc.sync.dma_start(qsh[64:96, :], qk_t[m][96:128, :])
                    nc.sync.dma_start(qsh[96:127, :], qk_t[m][64:95, :])
                    nc.sync.dma_start(qsh[127:128, :], fork[0:1, :])
                    nc.vector.tensor_mul(qk_t[m][:], qk_t[m][:], cos_sb[:])
                    nc.vector.tensor_mul(qsh[:], qsh[:], nsin_sb[:])
                    nc.vector.tensor_add(qk_t[m][:], qk_t[m][:], qsh[:])

                with nc.named_scope("qk_proj"):
                    for m in range(4):
                        qk_group(m)

                with nc.named_scope("v_proj"):
                    for mt in range(NT):
                        vps = vpsp.tile([P, 256], F32, name=f"vps{mt}",
                                        tag="vps")
                        for k in range(NKT):
                            nc.tensor.matmul(
                                vps[:],
                                lhsT=xt[:, k * T + mt * P: k * T + (mt + 1) * P],
                                rhs=wv[:, k * 256:(k + 1) * 256],
                                start=(k == 0), stop=False)
                        nc.tensor.matmul(vps[:], lhsT=ones_sb[0:1, 0:P],
                                         rhs=bv_sb[0:1, :], start=False,
                                         stop=True)
                        vvs = vv[:, mt * 260:(mt + 1) * 260].rearrange(
                            "p (h x) -> p h x", x=65)
                        nc.vector.tensor_scalar_mul(
                            vvs[:, :, 0:64],
                            vps[:].rearrange("p (h x) -> p h x", x=64),
                            vs_sb[:, mt:mt + 1])
            pqsh.release()

            # ---- attention + deferred norm + output projection ----
            psps = tc.alloc_tile_pool(name="sps", bufs=2, space="PSUM")
            pyps = tc.alloc_tile_pool(name="yps", bufs=2, space="PSUM")
            ppps = tc.alloc_tile_pool(name="pps", bufs=2, space="PSUM")
            ppt = tc.alloc_tile_pool(name="ptp", bufs=4)
            pbc = tc.alloc_tile_pool(name="bcp", bufs=8)
            prcr = tc.alloc_tile_pool(name="rcrp", bufs=4)
            posb = tc.alloc_tile_pool(name="osbp", bufs=3)
            rcb_tiles = {}

            def attn_head(h, cch):
                """S^T -> exp -> mask -> PV for one (head, q-chunk)."""
                ti = h // 2
                ro = 64 * (h % 2)
                qt = qk_t[ti]
                kt = qk_t[2 + ti]
                qs = cch * CH
                nik = 4 * (cch + 1)
                yps = pyps.tile([65, CH], F32, name=f"yps_{h}_{cch}",
                                tag="yps")
                # score tiles: full pairs [512|512]; diagonal quad packed as
                # [512|384] then [256|128]
                units = []        # (pt_tile, [(ik, col_off, width), ...])
                for p2 in range(2 * cch):
                    units.append([(2 * p2, 0, CH), (2 * p2 + 1, CH, CH)])
                d = 4 * cch
                units.append([(d, 0, CH), (d + 1, CH, 384)])
                units.append([(d + 2, 0, 256), (d + 3, 256, P)])

                def s_unit(u):
                    spw = psps.tile([P, 2 * CH], F32,
                                    name=f"spw_{h}_{cch}_{u}", tag="sps")
                    pt = ppt.tile([P, 2 * CH], BF16,
                                  name=f"pt_{h}_{cch}_{u}", tag="pt")
                    parts = units[u]
                    w = parts[-1][1] + parts[-1][2]
                    for ik, off, width in parts:
                        nc.tensor.matmul(
                            spw[:, off:off + width],
                            lhsT=kt[ro:ro + 64, ik * P:(ik + 1) * P],
                            rhs=qt[ro:ro + 64, qs + (CH - width): qs + CH],
                            start=True, stop=True)
                    return spw, pt, w

                def e_unit(u, spw, pt, w):
                    nc.scalar.activation(pt[:, 0:w], spw[:, 0:w], AF.Exp,
                                         scale=SCALE)
                    if u == 2 * cch:
                        nc.vector.tensor_mul(pt[:, 0:896], pt[:, 0:896],
                                             dmA_sb[:])
                    elif u == 2 * cch + 1:
                        nc.vector.tensor_mul(pt[:, 0:384], pt[:, 0:384],
                                             dmB_sb[:])

                def v_unit(u, pt):
                    parts = units[u]
                    for ik, off, width in parts:
                        nc.tensor.matmul(
                            yps[:, CH - width:CH],
                            lhsT=vv[:, ik * 260 + h * 65: ik * 260 + h * 65 + 65],
                            rhs=pt[:, off:off + width],
                            start=(ik == 0), stop=(ik == nik - 1))

                nu = len(units)
                prev = None
                for u in range(nu):
                    spw, pt, w = s_unit(u)
                    e_unit(u, spw, pt, w)
                    if prev is not None:
                        v_unit(u - 1, prev)
                    prev = pt
                v_unit(nu - 1, prev)
                # drain: numerator rows -> yt; denominator -> 1/den row, then
                # Pool-broadcast to 64 partitions for the deferred norm mul
                nc.vector.tensor_copy(yt[ti][ro:ro + 64, qs:qs + CH],
                                      yps[0:64, :])
                rcr = prcr.tile([1, CH], BF16, name=f"rcr_{h}_{cch}",
                                tag="rcr")
                with nc.allow_low_precision(reason="bf16 softmax recip"):
                    nc.vector.reciprocal(rcr[0:1, :], yps[64:65, :])
                rcb = pbc.tile([P, CH], BF16, name=f"rcb_{cch}_{h}", tag="bc")
                rcb_tiles[(cch, h)] = rcb
                nc.gpsimd.partition_broadcast(rcb[:], rcr[0:1, :])

            def norm(cch):
                """deferred: yt[:, chunk] *= broadcast 1/den (already in rcb)."""
                qs = cch * CH
                for h in range(4):
                    ti, ro = h // 2, 64 * (h % 2)
                    rcb = rcb_tiles.pop((cch, h))
                    nc.vector.tensor_mul(yt[ti][ro:ro + 64, qs:qs + CH],
                                         yt[ti][ro:ro + 64, qs:qs + CH],
                                         rcb[ro:ro + 64, :])

            osb_tiles = {}

            def proj_piece(mt, n):
                pps = ppps.tile([P, CH], F32, name=f"pps{mt}_{n}", tag="pp")
                for kk in range(2):
                    nc.tensor.matmul(
                        pps[:],
                        lhsT=yt[kk][:, mt * P:(mt + 1) * P],
                        rhs=wp_sb[:, kk * C + n * CH: kk * C + (n + 1) * CH],
                        start=(kk == 0), stop=(kk == 1))
                if mt not in osb_tiles:
                    osb_tiles[mt] = posb.tile([P, C], BF16, name=f"osb{mt}",
                                              tag="osb")
                osb = osb_tiles[mt]
                nc.vector.tensor_copy(osb[:, n * CH:(n + 1) * CH], pps[:])
                if n == 1:
                    nc.sync.dma_start(out_d[mt * P:(mt + 1) * P, :], osb[:])

            with nc.named_scope("attn"):
                # chunk-outer; norm+proj of chunk c interleave into chunk c+1
                for cch in range(4):
                    pending = []
                    if cch > 0:
                        pending = [(mt, n) for mt in range(4 * (cch - 1), 4 * cch)
                                   for n in range(2)]
                    for h in range(4):
                        attn_head(h, cch)
                        if h == 0 and cch > 0:
                            norm(cch - 1)
                        if cch > 0 and h > 0:
                            for mt, n in pending[(h - 1) * 3:h * 3]:
                                proj_piece(mt, n)
                    if cch > 0:
                        for mt, n in pending[9:]:
                            proj_piece(mt, n)
                norm(3)
                for mt in range(12, NT):
                    for n in range(2):
                        proj_piece(mt, n)

            posb.release()
            prcr.release()
            pbc.release()
            ppt.release()
            ppps.release()
            pyps.release()
            psps.release()
    nc.compile()
    _NC_CACHE["nc"] = nc
    return nc


def make_in_maps(x, cumulative_scores, padding_mask, W_attn, b_attn, W_proj,
                 b_proj, token_index):
    x = np.asarray(x, dtype=np.float32)
    cs = np.asarray(cumulative_scores, dtype=np.float32)
    pm = np.asarray(padding_mask, dtype=np.float32)
    Wa = np.asarray(W_attn, dtype=np.float32)
    ba = np.asarray(b_attn, dtype=np.float32)
    Wp = np.asarray(W_proj, dtype=np.float32)
    tok = np.asarray(token_index).astype(np.int64)
    B = x.shape[0]

    # packed diagonal masks: per tile a [tri(128) | ones] block, widths
    # 512|384 (A) and 256|128 (B)
    tri = (np.arange(P)[None, :] >= np.arange(P)[:, None]).astype(np.float32)
    dmA = np.ones((P, 896), np.float32)
    dmA[:, 0:P] = tri
    dmA[:, 512:512 + P] = tri
    dmB = np.ones((P, 384), np.float32)
    dmB[:, 0:P] = tri
    dmB[:, 256:256 + P] = tri
    ones_row = np.ones((1, T), np.float32)

    per_batch = []
    for b in range(B):
        counts = np.bincount(tok[b], minlength=T).astype(np.float32)
        with np.errstate(divide="ignore"):
            invc = (1.0 / counts).astype(np.float32)
        partial = np.cumsum(invc[tok[b]], dtype=np.float32)
        invf = (1.0 / (10000.0 ** (np.arange(0, 64, 2, dtype=np.float32) / 64.0))
                ).astype(np.float32)
        ang = partial[:, None].astype(np.float32) * invf[None, :]
        cos32 = np.cos(ang).T.astype(np.float32)
        sin32 = np.sin(ang).T.astype(np.float32)
        cos128 = np.ascontiguousarray(np.tile(cos32, (4, 1)))
        # dest-indexed rotate-half signs; fork rows folded in
        nsin128 = np.ascontiguousarray(
            np.concatenate([-sin32, sin32, -sin32, sin32], axis=0))
        cos128[63, :] = 0.0
        cos128[127, :] = 0.0
        nsin128[63, :] = 1.0
        nsin128[127, :] = 1.0
        pmg = np.take_along_axis(pm[b], tok[b], axis=0).astype(np.float32)
        pmbin = (pmg != 0).astype(np.float32)
        vscale = (np.exp(cs[b]).astype(np.float32) * pmbin).astype(np.float32)
        per_batch.append({
            "xt": np.ascontiguousarray(x[b].T).astype(BF),
            "cos2": cos128.astype(BF),
            "nsin": nsin128.astype(BF),
            "csrow": np.ascontiguousarray(cs[b][None, :]).astype(BF),
            "vscale": np.ascontiguousarray(vscale.reshape(NT, P).T),
            "onec": np.ascontiguousarray(pmbin.reshape(NT, P).T).astype(BF),
        })

    in_maps = []
    for core in range(8):
        b = core // 4
        g = core % 4
        qc = slice(g * 256, (g + 1) * 256)
        kc = slice(C + g * 256, C + (g + 1) * 256)
        vc = slice(2 * C + g * 256, 2 * C + (g + 1) * 256)
        wqk = np.ascontiguousarray(np.concatenate([Wa[:, qc], Wa[:, kc]],
                                                  axis=1))
        bqk_flat = np.concatenate([ba[qc], ba[kc]])          # [512]
        bqk = np.ascontiguousarray(bqk_flat.reshape(4, P).T)
        in_maps.append({
            **per_batch[b],
            "wqk": wqk.astype(BF),
            "wv": np.ascontiguousarray(Wa[:, vc]).astype(BF),
            "wp": np.ascontiguousarray(Wp[g * 256:(g + 1) * 256, :]).astype(BF),
            "bqk": bqk,
            "bv": np.ascontiguousarray(ba[vc][None, :]).astype(BF),
            "dmaskA": dmA.astype(BF),
            "dmaskB": dmB.astype(BF),
            "ones": ones_row.astype(BF),
        })
    return in_maps


def kernel(x, cumulative_scores, padding_mask, W_attn, b_attn, W_proj, b_proj,
           token_index, _results_hook=None):
    nc = build_nc()
    in_maps = make_in_maps(x, cumulative_scores, padding_mask, W_attn, b_attn,
                           W_proj, b_proj, token_index)
    res = run_bass_kernel_spmd(nc, in_maps, list(range(8)))
    if _results_hook is not None:
        _results_hook(res)
    bp = np.asarray(b_proj, dtype=np.float32)
    B = np.asarray(x).shape[0]
    out = np.zeros((B, T, C), np.float32)
    for b in range(B):
        acc = np.zeros((T, C), np.float32)
        for g in range(4):
            acc += np.asarray(res.results[b * 4 + g]["outp"], dtype=np.float32)
        out[b] = acc + bp[None, :]
    return out
